# revision 53
# baseline (speedup 1.0000x reference)
"""Concordance-index loss on Trainium2 (8 NeuronCores, Bass/Tile).

Reference math over N=8192 samples (t = exp(event_time), d = event_indicator,
r = estimate), pairwise over ordered pairs (i, j):
    comp(i,j)  = d_i & (t_i < t_j | (t_i == t_j & ~d_j))
    conc       = sum comp & (r_j - r_i < 0)
    tied       = sum comp & |r_j - r_i| <= 1e-8
    total      = sum comp
    disc       = total - conc - tied
    out        = 1 - (disc + 0.5*tied) / (disc + conc + tied + 1e-7)

Device strategy (host does only O(N log N) re-encoding; the O(N_ev * N)
pairwise compares run on the NeuronCores):

 - comp has a d_i factor, so only event rows (d_i=1) can contribute to ANY
   count.  Censored i rows are dropped on host: the i axis shrinks to the
   ~N/2 event rows, halving the pairwise work.
 - t is quantized (0.05 grid in log space; exp is strictly monotone), so t is
   replaced by its dense rank.  The predicate (t_i < t_j | (t_i == t_j &
   ~d_j)) collapses to ONE compare  4*trk_i + 2 < 4*trk_j + 2*(1-d_j) + 1,
   with the left side even and the right side odd (never equal).
 - All compare operands are embedded as monotonically increasing bf16 bit
   patterns (0x2000 + small_int viewed as bf16), so order compares are exact
   and every tensor operand is 16-bit (fast DVE perf modes).
 - r is replaced by its dense rank: conc(i,j) = (rrk_i > rrk_j), which is
   exactly (r_j < r_i) including duplicate handling.
 - tied pairs (|fl(r_j - r_i)| <= 1e-8, same IEEE f32 subtract as the
   reference) are found on host by a two-pointer sweep over sorted r; for
   those O(few) pairs comp is evaluated exactly on host.  This removes two
   of the three masked pairwise passes from the device.

Sharding: each core gets every 8th event of the t-rank-sorted event list
(round-robin, so all cores' rank prefixes agree within one row) as its
free-dim i-slice; j loops over all 8192 as 64 partition-chunks of 128,
with j sorted by its threshold w_j = trk_j + (1-d_j).

Triangular structure: comp(i,j) = [trk_i < w_j], so along the trk-sorted
i axis every comp row is a prefix of ones.  Per chunk (128 consecutive
sorted j) only the leading fd columns (the chunk's max prefix, ~half the
slice on average) can be nonzero, so both instructions run on [0, fd)
and chunks with fd == 0 emit nothing.  Per live chunk TWO VectorE
instructions:
    tensor_scalar  comp = (tm_i <  u_j)     [4x DVE mode, NO accum]
    scalar_tensor_tensor (re_i > rj_j)*comp [1x]  accum -> conc
This streams ~17M lane-elements instead of 34M.  `total` is summed on
host from the exact per-j prefix counts: an accum_out on the 4x-mode
tensor_scalar measured ~1us/instruction (62us/pass, same-shape A/B),
while the 1x STT's accum is ~free.  Chunks run in groups of 8 (8
producers, then the 8 dependent consumers) so no instruction stalls on
the SBUF write of its immediate predecessor (removing a ~3x RAW-stall
penalty).  The host all-reduces the conc tile and applies the formula.

Rejected alternatives (measured slower on this part):
 - TensorE histogram-matmul (collapses the 34M pairs to ~3K matmul rows):
   correct but per-instruction overhead dominated at the small free dims
   (76us vs 51us per pass).
 - ScalarE Sign-offload of the comp pass: ScalarE is ~2.3x off spec
   (known TRN2 errata), became the critical path (94us).
 - Pool-engine chunk offload: neuronxcc rejects DVE-style ops on Pool here.
"""

import numpy as np

N = 8192
NCORES = 8
P = 128
CCH = N // P                # 64 j partition-chunks

_CACHE = {}


def _build_nc(free, fds, repeat=1):
    """free = i-slice width per core (multiple of 8).  fds[jc] = number of
    leading i columns chunk jc must process (0 = chunk provably empty, no
    instructions emitted): with the i axis sorted by t-rank and j sorted by
    its threshold w_j, comp[j, i] is a prefix of ones, so columns beyond the
    chunk's max prefix contribute 0 to both accumulations.  (A further split
    of the all-ones prefix into an unmasked 4x count measured SLOWER: the
    extra 64 small-FD instructions pay a ~400ns per-instruction floor that
    outweighs the streaming saved.)  repeat: loop the compute (outputs
    overwritten; used for slope timing)."""
    import concourse.bass as bass
    from concourse import mybir

    dt = mybir.dt
    Alu = mybir.AluOpType

    nc = bass.Bass()
    # All inputs byte-packed into ONE dram tensor (bf16 tm|re broadcast rows,
    # then f32 uj|rj scalars) so the whole kernel uses exactly two DMAs.
    NB16 = 2 * free * 2                 # bytes of bf16 payload per partition
    NB32 = 2 * CCH * 4                  # bytes of f32 payload per partition
    xin = nc.declare_dram_parameter("xin", [P, NB16 + NB32], dt.uint8,
                                    isOutput=False)
    out = nc.declare_dram_parameter("out", [P, CCH], dt.float32,
                                    isOutput=True)

    with (
        nc.sbuf_tensor([P, NB16 + NB32], dt.uint8) as xin_s,
        nc.sbuf_tensor([P, CCH], dt.float32) as out_s,
        nc.sbuf_tensor([P, 16 * free], dt.bfloat16) as comp,
        nc.sbuf_tensor([P, free], dt.bfloat16) as dead,
        nc.semaphore() as dsem,
        nc.semaphore() as vsem,
        nc.Block() as block,
    ):
        x16_s = xin_s[:, 0:NB16].bitcast(dt.bfloat16)
        xf32_s = xin_s[:, NB16:NB16 + NB32].bitcast(dt.float32)
        tmr_s = x16_s[:, 0:free]
        rke_s = x16_s[:, free:2 * free]
        uj_s = xf32_s[:, 0 * CCH:1 * CCH]
        rj_s = xf32_s[:, 1 * CCH:2 * CCH]
        cc_s = out_s[:, 0:CCH]

        @block.gpsimd
        def _(g):
            g.dma_start(xin_s[:], xin[:]).then_inc(dsem, 16)
            g.wait_ge(vsem, 1)
            g.dma_start(out[:], out_s[:]).then_inc(dsem, 16)

        # The mask tensor_scalar carries NO accum_out: an accumulator on
        # the 4x-mode TS measured ~1us/instruction (62us/pass A/B, same
        # shape +-flag), so `total` is summed on host from the exact
        # per-j prefix counts instead.  The 1x STT keeps its accum_out
        # (replacing it with product tiles + tensor_tensor adds measured
        # slower in A/B).
        @block.vector
        def _(v):
            v.wait_ge(dsem, 16)
            last = None
            for _rep in range(repeat):
                # Chunks in groups of 16: all 16 comp tiles are produced
                # before any is consumed, so the STT never stalls on the
                # SBUF write-ack of its producer even where the early
                # chunks' tiny free-dims make instructions very short.
                for jg in range(0, CCH, 16):
                    for jc in range(jg, jg + 16):
                        b = jc - jg
                        fd = fds[jc]
                        if fd == 0:
                            continue
                        v.tensor_scalar(
                            comp[:, b * free:b * free + fd], tmr_s[:, 0:fd],
                            uj_s[:, jc:jc + 1], None, Alu.is_lt,
                        )
                    for jc in range(jg, jg + 16):
                        b = jc - jg
                        fd = fds[jc]
                        if fd == 0:
                            continue
                        last = v.scalar_tensor_tensor(
                            dead[:, 0:fd], rke_s[:, 0:fd],
                            rj_s[:, jc:jc + 1],
                            comp[:, b * free:b * free + fd],
                            op0=Alu.is_gt, op1=Alu.mult,
                            accum_out=cc_s[:, jc:jc + 1],
                        )
            last.then_inc(vsem, 1)

    return nc


def _embed(v):
    """Small non-negative ints -> monotone increasing positive bf16 values
    (bit patterns 0x2000+v), returned as uint16 bit patterns."""
    v = np.asarray(v, dtype=np.int64)
    assert v.min() >= 0 and v.max() < 24000
    return (0x2000 + v).astype(np.uint16)


def _emb_f32(bits):
    """f32 value of bf16 bit patterns (exact)."""
    return (bits.astype(np.uint32) << 16).view(np.float32)


def _host_tied(d, trk, r):
    """Exact tied count: pairs with |fl(r_x - r_y)| <= 1e-8f (the reference's
    f32 subtract), each direction gated by comp.  Two-pointer over sorted r."""
    n = r.shape[0]
    o = np.argsort(r, kind="stable")
    rs = r[o].astype(np.float32)
    thr = np.float32(1e-8)
    tied = 0.0
    lo = 0
    for k in range(n):
        while np.abs(rs[k] - rs[lo]) > thr:
            lo += 1
        for m in range(lo, k):
            x, y = o[k], o[m]
            # comp(x,y): d_x & (t_x < t_y | (t_x == t_y & ~d_y))
            if d[x] and (trk[x] < trk[y] or (trk[x] == trk[y] and not d[y])):
                tied += 1.0
            if d[y] and (trk[y] < trk[x] or (trk[y] == trk[x] and not d[x])):
                tied += 1.0
    return tied


def _prep_inputs(event_indicator, event_time, estimate):
    d = np.asarray(event_indicator).reshape(-1).astype(bool)
    t = np.asarray(event_time, dtype=np.float32).reshape(-1)
    r = np.asarray(estimate, dtype=np.float32).reshape(-1)
    n = t.shape[0]
    assert n == N

    # t dense ranks (exp is strictly increasing: ranking raw log-times
    # preserves order and equality of t = exp(event_time)).
    tv = np.unique(t)
    trk = np.searchsorted(tv, t).astype(np.int64)
    # Integer codes: tm = 4*trk + 2 (even), u = 4*trk + 2*(1-d) + 1 (odd).
    # tm < u  <=>  trk_i < trk_j | (trk_i == trk_j & ~d_j).
    ucode = 4 * trk + 2 * (1 - d.astype(np.int64)) + 1
    tmcode = 4 * trk + 2
    sent = 4 * len(tv) + 8          # > every ucode

    # r dense ranks: conc(i,j) = rrk_i > rrk_j  ==  r_j < r_i exactly.
    rv = np.unique(r)
    rrk = np.searchsorted(rv, r).astype(np.int64)

    tm_e = _embed(tmcode)
    u_e = _embed(ucode)
    re_e = _embed(rrk)
    sent_e = _embed(np.array([sent]))[0]

    # Event rows only; pad each core's slice with sentinel rows (comp = 0).
    ev = np.flatnonzero(d)
    nev = len(ev)
    percore = -(-nev // NCORES)
    free = max(32, -(-percore // 8) * 8)
    tied = _host_tied(d, trk, r)

    # Triangular pruning: events sorted by trk, dealt round-robin to cores
    # (so every core's prefix counts match within 1); j sorted by w = trk +
    # (1-d), so each 128-j chunk only needs its leading max-prefix columns.
    ev = ev[np.argsort(trk[ev], kind="stable")]
    w = trk + (1 - d.astype(np.int64))
    jperm = np.argsort(w, kind="stable")

    def jscalar(bits):
        # sorted-j position jc*128 + p -> element [p, jc] of [128, 64] f32
        return np.ascontiguousarray(
            _emb_f32(bits[jperm]).reshape(CCH, P).T.astype(np.float32))

    uj = jscalar(u_e)
    rj = jscalar(re_e)
    xf32 = np.ascontiguousarray(np.concatenate([uj, rj], axis=1))
    b32 = xf32.view(np.uint8).reshape(P, -1)

    # per-chunk free-dim bound: a core's slice has at most ceil(p/8) events
    # below a global threshold with p global events under it
    trk_sorted_ev = trk[ev]
    wmax = w[jperm].reshape(CCH, P).max(axis=1)
    pmax = np.searchsorted(trk_sorted_ev, wmax, side="left")
    fds = []
    for phi in pmax:
        pc = -(-int(phi) // NCORES)
        fds.append(int(min(free, -(-pc // 8) * 8)))

    # total decouples: sum over j of the exact per-j event-prefix count
    # (the device's accumulator costs ~1us/instruction, so the mask
    # tensor_scalar no longer carries accum_out and total comes from here)
    total = int(np.searchsorted(trk_sorted_ev, w, side="left").sum())

    in_maps = []
    for c in range(NCORES):
        idx = ev[c::NCORES]
        tmrow = np.full(free, sent_e, dtype=np.uint16)
        rerow = np.full(free, _embed(np.array([0]))[0], dtype=np.uint16)
        tmrow[:len(idx)] = tm_e[idx]
        rerow[:len(idx)] = re_e[idx]
        row16 = np.concatenate([tmrow, rerow]).view("<u2")
        b16 = np.ascontiguousarray(
            np.broadcast_to(row16[None, :], (P, 2 * free))).view(np.uint8)
        in_maps.append({
            "xin": np.ascontiguousarray(np.concatenate([b16, b32], axis=1)),
        })
    aux = {"tied": tied, "free": free, "fds": tuple(fds), "total": total}
    return in_maps, free, aux


def _finish(results, aux):
    live = np.asarray([fd > 0 for fd in aux["fds"]], dtype=np.float64)
    tot = np.float64(aux["total"])
    conc = np.float64(0.0)
    for res in results:
        # columns of skipped (provably-empty) chunks hold uninitialized SBUF
        conc += (res["out"].astype(np.float64)[:, 0:CCH] * live).sum()
    tied = aux["tied"]
    disc = tot - conc - tied
    loss = (disc + 0.5 * tied) / (disc + conc + tied + 1e-7)
    return np.asarray(1.0 - loss, dtype=np.float32)


def kernel(event_indicator, event_time, estimate):
    from concourse.bass_utils import run_bass_kernel_spmd

    in_maps, free, aux = _prep_inputs(event_indicator, event_time, estimate)
    if not any(aux["fds"]):             # no comparable pairs at all
        tied = aux["tied"]
        loss = (0.5 * tied) / (tied + 1e-7)
        return np.asarray(1.0 - loss, dtype=np.float32)
    key = ("nc", free, aux["fds"])
    if key not in _CACHE:
        _CACHE[key] = _build_nc(free, aux["fds"])
    nc = _CACHE[key]
    out = run_bass_kernel_spmd(nc, in_maps, core_ids=list(range(NCORES)))
    return _finish(out.results, aux)
